# revision 3
# baseline (speedup 1.0000x reference)
"""Trainium2 Bass kernel for nn_Attention_39934605918652.

res[b] = W0 @ x0[b] + sum_{n=1..N-1} W2 @ tanh(W1a @ x0[b] + W1b @ x[b,n])

Key algebraic optimization: W2 does not depend on n, so
    sum_n W2 @ tanh(...) = W2 @ (sum_n tanh(...))
which removes the second big matmul (only a [B,H]x[H,F] remains).

Sharding: data-parallel over batch B=128 across 8 cores (16 batches/core),
weights replicated. No collectives.

Device layout (per core), everything f-major so the contraction dim sits on
SBUF partitions:
  xiT   [F=512, BL*256]  columns grouped 256 per batch (255 real + 1 zero pad)
  x0T   [F=512, BL=16]
  w1bT  [F=512, H=1024]  (= W1[:, F:].T)
  w1aT  [F=512, H=1024]  (= W1[:, :F].T)
  w2T   [H=1024, F=512]  (= W2.T)
  w0T   [F=512, F=512]   (= W0.T)
Output resT [F=512, BL=16] per core; host transposes + concatenates.
"""

import os
import numpy as np
from contextlib import ExitStack

import concourse.bass as bass
import concourse.tile as tile
from concourse import bacc, mybir
from concourse.bass_utils import run_bass_kernel_spmd

N_CORES = 8
B, N, F, H = 128, 256, 512, 1024
BL = B // N_CORES          # 16 batches per core
NI = N - 1                 # 255 real columns per batch
NP = 256                   # padded columns per batch
NF = F // 128              # 4 f-chunks
NH = H // 128              # 8 h-tiles
QUADS = BL // 4            # 4 batch-quads; per quad psum tile [128, 4*256]

F32 = mybir.dt.float32
F32R = mybir.dt.float32r

USE_F32R = os.environ.get("KB_NO_F32R", "") == ""


def _build_kernel():
    nc = bacc.Bacc(
        "TRN2", target_bir_lowering=False, debug=False, num_devices=N_CORES
    )

    MMDT = F32R if USE_F32R else F32
    xiT = nc.dram_tensor("xiT", [F, BL * NP], MMDT, kind="ExternalInput").ap()
    x0T = nc.dram_tensor("x0T", [F, BL], F32, kind="ExternalInput").ap()
    w1bT = nc.dram_tensor("w1bT", [F, H], MMDT, kind="ExternalInput").ap()
    w1aT = nc.dram_tensor("w1aT", [F, H], F32, kind="ExternalInput").ap()
    w2T = nc.dram_tensor("w2T", [H, F], F32, kind="ExternalInput").ap()
    w0T = nc.dram_tensor("w0T", [F, F], F32, kind="ExternalInput").ap()
    resT = nc.dram_tensor("resT", [F, BL], F32, kind="ExternalOutput").ap()

    with tile.TileContext(nc) as tc:
        with ExitStack() as ctx:
            _kernel_body(ctx, tc, xiT, x0T, w1bT, w1aT, w2T, w0T, resT)

    nc.compile()
    return nc


def _kernel_body(ctx, tc, xiT, x0T, w1bT, w1aT, w2T, w0T, resT):
    nc = tc.nc
    Tanh = mybir.ActivationFunctionType.Tanh

    wpool = ctx.enter_context(tc.tile_pool(name="weights", bufs=1))
    # Persistent SBUF tensors. Distinct tags so each gets its own slot.
    x0_sb = []
    for f in range(NF):
        t = wpool.tile([128, BL], F32, tag=f"x0_{f}", name=f"x0_{f}")
        nc.sync.dma_start(t[:], x0T[f * 128 : (f + 1) * 128, :])
        x0_sb.append(t)
    w1a_sb = []
    for f in range(NF):
        t = wpool.tile([128, H], F32, tag=f"w1a_{f}", name=f"w1a_{f}")
        nc.sync.dma_start(t[:], w1aT[f * 128 : (f + 1) * 128, :])
        w1a_sb.append(t)
    MMDT = F32R if USE_F32R else F32
    w1b_sb = []
    for f in range(NF):
        t = wpool.tile([128, H], MMDT, tag=f"w1b_{f}", name=f"w1b_{f}")
        nc.sync.dma_start(t[:], w1bT[f * 128 : (f + 1) * 128, :])
        w1b_sb.append(t)
    xi_sb = []
    for f in range(NF):
        t = wpool.tile([128, BL * NP], MMDT, tag=f"xi_{f}", name=f"xi_{f}")
        nc.sync.dma_start(t[:], xiT[f * 128 : (f + 1) * 128, :])
        xi_sb.append(t)
    w2_sb = []
    for h in range(NH):
        t = wpool.tile([128, F], F32, tag=f"w2_{h}", name=f"w2_{h}")
        nc.sync.dma_start(t[:], w2T[h * 128 : (h + 1) * 128, :])
        w2_sb.append(t)
    w0_sb = []
    for f in range(NF):
        t = wpool.tile([128, F], F32, tag=f"w0_{f}", name=f"w0_{f}")
        nc.sync.dma_start(t[:], w0T[f * 128 : (f + 1) * 128, :])
        w0_sb.append(t)

    h0_sb = [
        wpool.tile([128, BL], F32, tag=f"h0_{h}", name=f"h0_{h}")
        for h in range(NH)
    ]
    S_sb = [
        wpool.tile([128, BL], F32, tag=f"S_{h}", name=f"S_{h}")
        for h in range(NH)
    ]

    # One PSUM pool; every tile shares the tag so slots recycle.
    # Slot size = max tile = [128, 4*NP] f32 = 2 banks; 4 bufs = 8 banks.
    ppool = ctx.enter_context(tc.tile_pool(name="ps", bufs=4, space="PSUM"))
    # Interact scratch (tanh output) pool.
    itpool = ctx.enter_context(tc.tile_pool(name="it", bufs=4))

    # ---- Phase 1: h0[h, b] = sum_f W1a[h, f] * x0[b, f] (exact fp32) ----
    for h in range(NH):
        ph = ppool.tile([128, BL], F32, tag="ps", name=f"ph0_{h}")
        for f in range(NF):
            nc.tensor.matmul(
                ph[:],
                w1a_sb[f][:, h * 128 : (h + 1) * 128],
                x0_sb[f][:],
                start=(f == 0),
                stop=(f == NF - 1),
            )
        nc.vector.tensor_copy(h0_sb[h][:], ph[:])

    # ---- Phase 2: hi matmul + bias + tanh + segmented reduce ----
    for h in range(NH):
        for q in range(QUADS):
            pb = ppool.tile([128, 4 * NP], F32, tag="ps", name=f"pb_{h}_{q}")
            for f in range(NF):
                lhsT = w1b_sb[f][:, h * 128 : (h + 1) * 128]
                for bk in range(2):
                    cols = slice(q * 4 * NP + bk * 512, q * 4 * NP + (bk + 1) * 512)
                    nc.tensor.matmul(
                        pb[:, bk * 512 : (bk + 1) * 512],
                        lhsT,
                        xi_sb[f][:, cols],
                        start=(f == 0),
                        stop=(f == NF - 1),
                    )
            # add h0[:, b] to each batch's 255 real columns (pad col stays 0)
            for bl in range(4):
                b = q * 4 + bl
                sl = pb[:, bl * NP : bl * NP + NI]
                nc.vector.tensor_scalar_add(sl, sl, h0_sb[h][:, b : b + 1])
            it = itpool.tile([128, 4 * NP], F32, tag="it", name=f"it_{h}_{q}")
            nc.scalar.activation(it[:], pb[:], Tanh)
            # S[:, q*4+bl] = sum over the 256 columns of each batch block
            nc.vector.reduce_sum(
                S_sb[h][:, q * 4 : (q + 1) * 4],
                it[:].rearrange("p (b n) -> p b n", b=4),
                axis=mybir.AxisListType.X,
            )

    # ---- Phase 3: resT[g, b] = sum_h W2[g,h] S[h,b] + sum_f W0[g,f] x0[b,f] ----
    for ft in range(NF):
        po = ppool.tile([128, BL], F32, tag="ps", name=f"po_{ft}")
        for h in range(NH):
            nc.tensor.matmul(
                po[:],
                w2_sb[h][:, ft * 128 : (ft + 1) * 128],
                S_sb[h][:],
                start=(h == 0),
                stop=False,
            )
        for f in range(NF):
            nc.tensor.matmul(
                po[:],
                w0_sb[f][:, ft * 128 : (ft + 1) * 128],
                x0_sb[f][:],
                start=False,
                stop=(f == NF - 1),
            )
        rt = itpool.tile([128, BL], F32, tag="rt", name=f"rt_{ft}")
        nc.vector.tensor_copy(rt[:], po[:])
        nc.sync.dma_start(resT[ft * 128 : (ft + 1) * 128, :], rt[:])


_NC_CACHE = {}


def _get_nc():
    key = ("v1", USE_F32R)
    if key not in _NC_CACHE:
        _NC_CACHE[key] = _build_kernel()
    return _NC_CACHE[key]


def _make_in_maps(x, W1, W2, W0):
    x = np.ascontiguousarray(np.asarray(x, dtype=np.float32))
    W1 = np.asarray(W1, dtype=np.float32)
    W2 = np.asarray(W2, dtype=np.float32)
    W0 = np.asarray(W0, dtype=np.float32)

    w1aT = np.ascontiguousarray(W1[:, :F].T)  # [F, H]
    w1bT = np.ascontiguousarray(W1[:, F:].T)  # [F, H]
    w2T = np.ascontiguousarray(W2.T)          # [H, F]
    w0T = np.ascontiguousarray(W0.T)          # [F, F]

    in_maps = []
    for i in range(N_CORES):
        xc = x[i * BL : (i + 1) * BL]               # [BL, N, F]
        x0T = np.ascontiguousarray(xc[:, 0, :].T)   # [F, BL]
        pad = np.zeros((BL, NP, F), dtype=np.float32)
        pad[:, :NI, :] = xc[:, 1:, :]
        xiT = np.ascontiguousarray(pad.reshape(BL * NP, F).T)  # [F, BL*NP]
        in_maps.append(
            {
                "xiT": xiT,
                "x0T": x0T,
                "w1bT": w1bT,
                "w1aT": w1aT,
                "w2T": w2T,
                "w0T": w0T,
            }
        )
    return in_maps


def _gather(results):
    out = np.empty((B, F), dtype=np.float32)
    for i in range(N_CORES):
        out[i * BL : (i + 1) * BL] = results[i]["resT"].T
    return out


def kernel(x, W1, W2, W0):
    nc = _get_nc()
    in_maps = _make_in_maps(x, W1, W2, W0)
    res = run_bass_kernel_spmd(nc, in_maps, list(range(N_CORES)))
    return _gather(res.results)


def kernel_profiled(x, W1, W2, W0, **trace_kwargs):
    """Like kernel() but with NTFF profiling; returns (out, exec_time_ns)."""
    nc = _get_nc()
    in_maps = _make_in_maps(x, W1, W2, W0)
    res = run_bass_kernel_spmd(
        nc, in_maps, list(range(N_CORES)), trace=True, **trace_kwargs
    )
    return _gather(res.results), res.exec_time_ns


# revision 6
# speedup vs baseline: 1.1533x; 1.1533x over previous
"""Trainium2 Bass kernel for nn_Attention_39934605918652.

res[b] = W0 @ x0[b] + sum_{n=1..N-1} W2 @ tanh(W1a @ x0[b] + W1b @ x[b,n])

Key algebraic optimization: W2 does not depend on n, so
    sum_n W2 @ tanh(...) = W2 @ (sum_n tanh(...))
which removes the second big matmul (only a [B,H]x[H,F] remains).

Sharding: data-parallel over batch B=128 across 8 cores (16 batches/core),
weights replicated. No collectives.

Device layout (per core), everything f-major so the contraction dim sits on
SBUF partitions:
  xiT   [F=512, BL*256]  columns grouped 256 per batch (255 real + 1 zero pad)
  x0T   [F=512, BL=16]
  w1bT  [F=512, H=1024]  (= W1[:, F:].T)
  w1aT  [F=512, H=1024]  (= W1[:, :F].T)
  w2T   [H=1024, F=512]  (= W2.T)
  w0T   [F=512, F=512]   (= W0.T)
Output resT [F=512, BL=16] per core; host transposes + concatenates.
"""

import os
import numpy as np
from contextlib import ExitStack

import concourse.bass as bass
import concourse.tile as tile
from concourse import bacc, mybir
from concourse.bass_utils import run_bass_kernel_spmd

N_CORES = 8
B, N, F, H = 128, 256, 512, 1024
BL = B // N_CORES          # 16 batches per core
NI = N - 1                 # 255 real columns per batch
NP = 256                   # padded columns per batch
NF = F // 128              # 4 f-chunks
NH = H // 128              # 8 h-tiles
QUADS = BL // 4            # 4 batch-quads; per quad psum tile [128, 4*256]

F32 = mybir.dt.float32
F32R = mybir.dt.float32r

USE_F32R = os.environ.get("KB_NO_F32R", "") == ""


def _build_kernel():
    nc = bacc.Bacc(
        "TRN2", target_bir_lowering=False, debug=False, num_devices=N_CORES
    )

    MMDT = F32R if USE_F32R else F32
    xiT = nc.dram_tensor("xiT", [F, BL * NP], MMDT, kind="ExternalInput").ap()
    x0T = nc.dram_tensor("x0T", [F, BL], F32, kind="ExternalInput").ap()
    w1bT = nc.dram_tensor("w1bT", [F, H], MMDT, kind="ExternalInput").ap()
    w1aT = nc.dram_tensor("w1aT", [F, H], F32, kind="ExternalInput").ap()
    w2T = nc.dram_tensor("w2T", [H, F], F32, kind="ExternalInput").ap()
    w0T = nc.dram_tensor("w0T", [F, F], F32, kind="ExternalInput").ap()
    resT = nc.dram_tensor("resT", [F, BL], F32, kind="ExternalOutput").ap()

    with tile.TileContext(nc) as tc:
        with ExitStack() as ctx:
            _kernel_body(ctx, tc, xiT, x0T, w1bT, w1aT, w2T, w0T, resT)

    nc.compile()
    return nc


def _kernel_body(ctx, tc, xiT, x0T, w1bT, w1aT, w2T, w0T, resT):
    nc = tc.nc
    Tanh = mybir.ActivationFunctionType.Tanh

    wpool = ctx.enter_context(tc.tile_pool(name="weights", bufs=1))
    # Persistent SBUF tensors. Distinct tags so each gets its own slot.
    x0_sb = []
    for f in range(NF):
        t = wpool.tile([128, BL], F32, tag=f"x0_{f}", name=f"x0_{f}")
        nc.sync.dma_start(t[:], x0T[f * 128 : (f + 1) * 128, :])
        x0_sb.append(t)
    w1a_sb = []
    for f in range(NF):
        t = wpool.tile([128, H], F32, tag=f"w1a_{f}", name=f"w1a_{f}")
        nc.sync.dma_start(t[:], w1aT[f * 128 : (f + 1) * 128, :])
        w1a_sb.append(t)
    MMDT = F32R if USE_F32R else F32
    w1b_sb = []
    for f in range(NF):
        t = wpool.tile([128, H], MMDT, tag=f"w1b_{f}", name=f"w1b_{f}")
        nc.sync.dma_start(t[:], w1bT[f * 128 : (f + 1) * 128, :])
        w1b_sb.append(t)
    xi_sb = []
    for f in range(NF):
        t = wpool.tile([128, BL * NP], MMDT, tag=f"xi_{f}", name=f"xi_{f}")
        nc.sync.dma_start(t[:], xiT[f * 128 : (f + 1) * 128, :])
        xi_sb.append(t)
    w2_sb = []
    for h in range(NH):
        t = wpool.tile([128, F], F32, tag=f"w2_{h}", name=f"w2_{h}")
        nc.sync.dma_start(t[:], w2T[h * 128 : (h + 1) * 128, :])
        w2_sb.append(t)
    w0_sb = []
    for f in range(NF):
        t = wpool.tile([128, F], F32, tag=f"w0_{f}", name=f"w0_{f}")
        nc.sync.dma_start(t[:], w0T[f * 128 : (f + 1) * 128, :])
        w0_sb.append(t)

    h0_sb = [
        wpool.tile([128, BL], F32, tag=f"h0_{h}", name=f"h0_{h}")
        for h in range(NH)
    ]
    S_sb = [
        wpool.tile([128, BL], F32, tag=f"S_{h}", name=f"S_{h}")
        for h in range(NH)
    ]

    # One PSUM pool; every tile shares the tag so slots recycle.
    # Slot size = max tile = [128, 4*NP] f32 = 2 banks; 4 bufs = 8 banks.
    ppool = ctx.enter_context(tc.tile_pool(name="ps", bufs=4, space="PSUM"))
    # Interact scratch (tanh output) pool.
    itpool = ctx.enter_context(tc.tile_pool(name="it", bufs=4))

    # ---- Phase 1: h0[h, b] = sum_f W1a[h, f] * x0[b, f] (exact fp32) ----
    for h in range(NH):
        ph = ppool.tile([128, BL], F32, tag="ps", name=f"ph0_{h}")
        for f in range(NF):
            nc.tensor.matmul(
                ph[:],
                w1a_sb[f][:, h * 128 : (h + 1) * 128],
                x0_sb[f][:],
                start=(f == 0),
                stop=(f == NF - 1),
            )
        nc.vector.tensor_copy(h0_sb[h][:], ph[:])

    # ---- Phase 2: hi matmul + bias + tanh + segmented reduce ----
    # 13 of the 32 quad-tiles go down the ACT path (bias+tanh+sum fused in
    # per-batch activation calls); the rest go down the DVE path (bias via
    # tensor_scalar into SBUF, one big tanh, strided reduce). This balances
    # ACT ~= DVE ~= 48us, both under the PE's ~55us.
    N_ACT_PATH = 13

    def act_path(idx):
        return (idx * N_ACT_PATH) // 32 != ((idx + 1) * N_ACT_PATH) // 32

    def consume(h, q, pb):
        if act_path(h * QUADS + q):
            for bl in range(4):
                b = q * 4 + bl
                scratch = itpool.tile(
                    [128, NI], F32, tag="scratch", name=f"sc_{h}_{q}_{bl}"
                )
                nc.scalar.activation(
                    scratch[:],
                    pb[:, bl * NP : bl * NP + NI],
                    Tanh,
                    bias=h0_sb[h][:, b : b + 1],
                    accum_out=S_sb[h][:, b : b + 1],
                )
        else:
            it = itpool.tile([128, 4 * NP], F32, tag="it", name=f"it_{h}_{q}")
            for bl in range(4):
                b = q * 4 + bl
                nc.vector.tensor_scalar_add(
                    it[:, bl * NP : bl * NP + NI],
                    pb[:, bl * NP : bl * NP + NI],
                    h0_sb[h][:, b : b + 1],
                )
            view = it[:].rearrange("p (b n) -> p b n", b=4)[:, :, :NI]
            nc.scalar.activation(view, view, Tanh)
            nc.vector.reduce_sum(
                S_sb[h][:, q * 4 : (q + 1) * 4],
                view,
                axis=mybir.AxisListType.X,
            )

    for h in range(NH):
        for wave in range(QUADS // 2):
            qs = (2 * wave, 2 * wave + 1)
            pbs = {
                q: ppool.tile([128, 4 * NP], F32, tag="ps", name=f"pb_{h}_{q}")
                for q in qs
            }
            for f in range(NF):
                lhsT = w1b_sb[f][:, h * 128 : (h + 1) * 128]
                for q in qs:
                    for bk in range(2):
                        cols = slice(
                            q * 4 * NP + bk * 512, q * 4 * NP + (bk + 1) * 512
                        )
                        nc.tensor.matmul(
                            pbs[q][:, bk * 512 : (bk + 1) * 512],
                            lhsT,
                            xi_sb[f][:, cols],
                            start=(f == 0),
                            stop=(f == NF - 1),
                        )
            for q in qs:
                consume(h, q, pbs[q])

    # ---- Phase 3: resT[g, b] = sum_h W2[g,h] S[h,b] + sum_f W0[g,f] x0[b,f] ----
    for ft in range(NF):
        po = ppool.tile([128, BL], F32, tag="ps", name=f"po_{ft}")
        for h in range(NH):
            nc.tensor.matmul(
                po[:],
                w2_sb[h][:, ft * 128 : (ft + 1) * 128],
                S_sb[h][:],
                start=(h == 0),
                stop=False,
            )
        for f in range(NF):
            nc.tensor.matmul(
                po[:],
                w0_sb[f][:, ft * 128 : (ft + 1) * 128],
                x0_sb[f][:],
                start=False,
                stop=(f == NF - 1),
            )
        rt = itpool.tile([128, BL], F32, tag="rt", name=f"rt_{ft}")
        nc.vector.tensor_copy(rt[:], po[:])
        nc.sync.dma_start(resT[ft * 128 : (ft + 1) * 128, :], rt[:])


_NC_CACHE = {}


def _get_nc():
    key = ("v2", USE_F32R)
    if key not in _NC_CACHE:
        _NC_CACHE[key] = _build_kernel()
    return _NC_CACHE[key]


def _make_in_maps(x, W1, W2, W0):
    x = np.ascontiguousarray(np.asarray(x, dtype=np.float32))
    W1 = np.asarray(W1, dtype=np.float32)
    W2 = np.asarray(W2, dtype=np.float32)
    W0 = np.asarray(W0, dtype=np.float32)

    w1aT = np.ascontiguousarray(W1[:, :F].T)  # [F, H]
    w1bT = np.ascontiguousarray(W1[:, F:].T)  # [F, H]
    w2T = np.ascontiguousarray(W2.T)          # [H, F]
    w0T = np.ascontiguousarray(W0.T)          # [F, F]

    in_maps = []
    for i in range(N_CORES):
        xc = x[i * BL : (i + 1) * BL]               # [BL, N, F]
        x0T = np.ascontiguousarray(xc[:, 0, :].T)   # [F, BL]
        pad = np.zeros((BL, NP, F), dtype=np.float32)
        pad[:, :NI, :] = xc[:, 1:, :]
        xiT = np.ascontiguousarray(pad.reshape(BL * NP, F).T)  # [F, BL*NP]
        in_maps.append(
            {
                "xiT": xiT,
                "x0T": x0T,
                "w1bT": w1bT,
                "w1aT": w1aT,
                "w2T": w2T,
                "w0T": w0T,
            }
        )
    return in_maps


def _gather(results):
    out = np.empty((B, F), dtype=np.float32)
    for i in range(N_CORES):
        out[i * BL : (i + 1) * BL] = results[i]["resT"].T
    return out


def kernel(x, W1, W2, W0):
    nc = _get_nc()
    in_maps = _make_in_maps(x, W1, W2, W0)
    res = run_bass_kernel_spmd(nc, in_maps, list(range(N_CORES)))
    return _gather(res.results)


def kernel_profiled(x, W1, W2, W0, **trace_kwargs):
    """Like kernel() but with NTFF profiling; returns (out, exec_time_ns)."""
    nc = _get_nc()
    in_maps = _make_in_maps(x, W1, W2, W0)
    res = run_bass_kernel_spmd(
        nc, in_maps, list(range(N_CORES)), trace=True, **trace_kwargs
    )
    return _gather(res.results), res.exec_time_ns


# revision 9
# speedup vs baseline: 1.2657x; 1.0975x over previous
"""Trainium2 Bass kernel for nn_Attention_39934605918652.

res[b] = W0 @ x0[b] + sum_{n=1..N-1} W2 @ tanh(W1a @ x0[b] + W1b @ x[b,n])

Key algebraic optimization: W2 does not depend on n, so
    sum_n W2 @ tanh(...) = W2 @ (sum_n tanh(...))
which removes the second big matmul (only a [B,H]x[H,F] remains).

Sharding: data-parallel over batch B=128 across 8 cores (16 batches/core),
weights replicated. No collectives.

All matmuls run as float32r (TF32-like, 1 cycle/row on the PE at N>=256
vs 4 for fp32); PSUM accumulation stays fp32. Measured end-to-end rel
err vs the fp64 oracle is ~1e-5..1e-4.

Device layout (per core), everything f-major so the contraction dim sits on
SBUF partitions:
  xiT   [F=512, BL*256]  columns grouped 256 per batch (255 real + 1 zero pad)
  x0T   [F=512, BL=16]
  w1bT  [F=512, H=1024]  (= W1[:, F:].T)
  w1aT  [F=512, H=1024]  (= W1[:, :F].T)
  w2T   [H=1024, F=512]  (= W2.T)
  w0T   [F=512, F=512]   (= W0.T)
Output res [BL=16, F=512] per core (batch-major); host concatenates.
"""

import os
import numpy as np
from contextlib import ExitStack

import concourse.bass as bass
import concourse.tile as tile
from concourse import bacc, mybir
from concourse.bass_utils import run_bass_kernel_spmd

N_CORES = 8
B, N, F, H = 128, 256, 512, 1024
BL = B // N_CORES          # 16 batches per core
NI = N - 1                 # 255 real columns per batch
NP = 256                   # padded columns per batch
NF = F // 128              # 4 f-chunks
NH = H // 128              # 8 h-tiles
QUADS = BL // 4            # 4 batch-quads; per quad psum tile [128, 4*256]

F32 = mybir.dt.float32
F32R = mybir.dt.float32r

USE_F32R = os.environ.get("KB_NO_F32R", "") == ""
# How many of the 32 quad-tiles take the fused-ACT consumer path
# (bias+tanh+sum in per-batch activation calls); the rest take the DVE
# path. Balances ACT ~= DVE busy time.
N_ACT_PATH = int(os.environ.get("KB_NACT", "10"))


def _build_kernel():
    nc = bacc.Bacc(
        "TRN2", target_bir_lowering=False, debug=False, num_devices=N_CORES
    )

    MMDT = F32R if USE_F32R else F32
    xiT = nc.dram_tensor("xiT", [F, BL * NP], MMDT, kind="ExternalInput").ap()
    x0T = nc.dram_tensor("x0T", [F, BL], MMDT, kind="ExternalInput").ap()
    w1bT = nc.dram_tensor("w1bT", [F, H], MMDT, kind="ExternalInput").ap()
    w1aT = nc.dram_tensor("w1aT", [F, H], MMDT, kind="ExternalInput").ap()
    w2T = nc.dram_tensor("w2T", [H, F], MMDT, kind="ExternalInput").ap()
    w0T = nc.dram_tensor("w0T", [F, F], MMDT, kind="ExternalInput").ap()
    res = nc.dram_tensor("res", [BL, F], F32, kind="ExternalOutput").ap()

    with tile.TileContext(nc) as tc:
        with ExitStack() as ctx:
            _kernel_body(ctx, tc, xiT, x0T, w1bT, w1aT, w2T, w0T, res)

    nc.compile()
    return nc


def _kernel_body(ctx, tc, xiT, x0T, w1bT, w1aT, w2T, w0T, res):
    nc = tc.nc
    Tanh = mybir.ActivationFunctionType.Tanh
    MMDT = F32R if USE_F32R else F32

    wpool = ctx.enter_context(tc.tile_pool(name="weights", bufs=1))

    def load(name, dram, rows, width, dt):
        tiles = []
        for c in range(rows // 128):
            t = wpool.tile([128, width], dt, tag=f"{name}_{c}", name=f"{name}_{c}")
            nc.sync.dma_start(t[:], dram[c * 128 : (c + 1) * 128, :])
            tiles.append(t)
        return tiles

    # DMA issue order = first-need order.
    x0_sb = load("x0", x0T, F, BL, MMDT)
    w1a_sb = load("w1a", w1aT, F, H, MMDT)
    w1b_sb = load("w1b", w1bT, F, H, MMDT)
    xi_sb = load("xi", xiT, F, BL * NP, MMDT)
    w2_sb = load("w2", w2T, H, F, MMDT)
    w0_sb = load("w0", w0T, F, F, MMDT)

    h0_sb = [
        wpool.tile([128, BL], F32, tag=f"h0_{h}", name=f"h0_{h}")
        for h in range(NH)
    ]
    S_sb = [
        wpool.tile([128, BL], MMDT, tag=f"S_{h}", name=f"S_{h}")
        for h in range(NH)
    ]

    # One PSUM pool; every tile shares the tag so slots recycle.
    # Slot size = max tile = [128, 4*NP] f32 = 2 banks; 4 bufs = 8 banks.
    ppool = ctx.enter_context(tc.tile_pool(name="ps", bufs=4, space="PSUM"))
    itpool = ctx.enter_context(tc.tile_pool(name="it", bufs=4))

    # ---- Phase 1: h0[h, b] = sum_f W1a[h, f] * x0[b, f] ----
    for h in range(NH):
        ph = ppool.tile([128, BL], F32, tag="ps", name=f"ph0_{h}")
        for f in range(NF):
            nc.tensor.matmul(
                ph[:],
                w1a_sb[f][:, h * 128 : (h + 1) * 128],
                x0_sb[f][:],
                start=(f == 0),
                stop=(f == NF - 1),
            )
        nc.vector.tensor_copy(h0_sb[h][:], ph[:])

    # ---- Phase 2: hi matmul + bias + tanh + segmented reduce ----
    def act_path(idx):
        return (idx * N_ACT_PATH) // 32 != ((idx + 1) * N_ACT_PATH) // 32

    def consume(h, q, pb):
        if act_path(h * QUADS + q):
            # fused: tanh(psum + h0) with free-dim sum into S, per batch
            for bl in range(4):
                b = q * 4 + bl
                scratch = itpool.tile(
                    [128, NI], F32, tag="scratch", name=f"sc_{h}_{q}_{bl}"
                )
                with nc.allow_low_precision(
                    reason="S in f32r so it can feed the f32r output matmul"
                ):
                    nc.scalar.activation(
                        scratch[:],
                        pb[:, bl * NP : bl * NP + NI],
                        Tanh,
                        bias=h0_sb[h][:, b : b + 1],
                        accum_out=S_sb[h][:, b : b + 1],
                    )
        else:
            it = itpool.tile([128, 4 * NP], F32, tag="it", name=f"it_{h}_{q}")
            for bl in range(4):
                b = q * 4 + bl
                nc.vector.tensor_scalar_add(
                    it[:, bl * NP : bl * NP + NI],
                    pb[:, bl * NP : bl * NP + NI],
                    h0_sb[h][:, b : b + 1],
                )
            view = it[:].rearrange("p (b n) -> p b n", b=4)[:, :, :NI]
            nc.scalar.activation(view, view, Tanh)
            with nc.allow_low_precision(
                reason="S in f32r so it can feed the f32r output matmul"
            ):
                nc.vector.reduce_sum(
                    S_sb[h][:, q * 4 : (q + 1) * 4],
                    view,
                    axis=mybir.AxisListType.X,
                )

    for h in range(NH):
        for wave in range(QUADS // 2):
            qs = (2 * wave, 2 * wave + 1)
            pbs = {
                q: ppool.tile([128, 4 * NP], F32, tag="ps", name=f"pb_{h}_{q}")
                for q in qs
            }
            for f in range(NF):
                lhsT = w1b_sb[f][:, h * 128 : (h + 1) * 128]
                for q in qs:
                    for bk in range(2):
                        cols = slice(
                            q * 4 * NP + bk * 512, q * 4 * NP + (bk + 1) * 512
                        )
                        nc.tensor.matmul(
                            pbs[q][:, bk * 512 : (bk + 1) * 512],
                            lhsT,
                            xi_sb[f][:, cols],
                            start=(f == 0),
                            stop=(f == NF - 1),
                        )
            for q in qs:
                consume(h, q, pbs[q])

    # ---- Phase 3 (flipped): res[b, g] = sum_h S[h,b] W2T[h,g]
    #                                   + sum_f x0T[f,b] W0T[f,g] ----
    # b (=16) is the PE's M dim; N=512 streams. 12 matmuls total.
    po = ppool.tile([BL, F], F32, tag="ps", name="po")
    for h in range(NH):
        nc.tensor.matmul(
            po[:], S_sb[h][:], w2_sb[h][:], start=(h == 0), stop=False
        )
    for f in range(NF):
        nc.tensor.matmul(
            po[:], x0_sb[f][:], w0_sb[f][:], start=False, stop=(f == NF - 1)
        )
    rt = itpool.tile([BL, F], F32, tag="rt", name="rt")
    nc.vector.tensor_copy(rt[:], po[:])
    nc.sync.dma_start(res[:], rt[:])


_NC_CACHE = {}


def _get_nc():
    key = ("v3", USE_F32R, N_ACT_PATH)
    if key not in _NC_CACHE:
        _NC_CACHE[key] = _build_kernel()
    return _NC_CACHE[key]


def _make_in_maps(x, W1, W2, W0):
    x = np.ascontiguousarray(np.asarray(x, dtype=np.float32))
    W1 = np.asarray(W1, dtype=np.float32)
    W2 = np.asarray(W2, dtype=np.float32)
    W0 = np.asarray(W0, dtype=np.float32)

    w1aT = np.ascontiguousarray(W1[:, :F].T)  # [F, H]
    w1bT = np.ascontiguousarray(W1[:, F:].T)  # [F, H]
    w2T = np.ascontiguousarray(W2.T)          # [H, F]
    w0T = np.ascontiguousarray(W0.T)          # [F, F]

    in_maps = []
    for i in range(N_CORES):
        xc = x[i * BL : (i + 1) * BL]               # [BL, N, F]
        x0T = np.ascontiguousarray(xc[:, 0, :].T)   # [F, BL]
        pad = np.zeros((BL, NP, F), dtype=np.float32)
        pad[:, :NI, :] = xc[:, 1:, :]
        xiT = np.ascontiguousarray(pad.reshape(BL * NP, F).T)  # [F, BL*NP]
        in_maps.append(
            {
                "xiT": xiT,
                "x0T": x0T,
                "w1bT": w1bT,
                "w1aT": w1aT,
                "w2T": w2T,
                "w0T": w0T,
            }
        )
    return in_maps


def _gather(results):
    out = np.empty((B, F), dtype=np.float32)
    for i in range(N_CORES):
        out[i * BL : (i + 1) * BL] = results[i]["res"]
    return out


def kernel(x, W1, W2, W0):
    nc = _get_nc()
    in_maps = _make_in_maps(x, W1, W2, W0)
    res = run_bass_kernel_spmd(nc, in_maps, list(range(N_CORES)))
    return _gather(res.results)


def kernel_profiled(x, W1, W2, W0, **trace_kwargs):
    """Like kernel() but with NTFF profiling; returns (out, exec_time_ns)."""
    nc = _get_nc()
    in_maps = _make_in_maps(x, W1, W2, W0)
    res = run_bass_kernel_spmd(
        nc, in_maps, list(range(N_CORES)), trace=True, **trace_kwargs
    )
    return _gather(res.results), res.exec_time_ns


# revision 13
# speedup vs baseline: 1.3142x; 1.0383x over previous
"""Trainium2 Bass kernel for nn_Attention_39934605918652.

res[b] = W0 @ x0[b] + sum_{n=1..N-1} W2 @ tanh(W1a @ x0[b] + W1b @ x[b,n])

Key algebraic optimization: W2 does not depend on n, so
    sum_n W2 @ tanh(...) = W2 @ (sum_n tanh(...))
which removes the second big matmul (only a [B,H]x[H,F] remains).

Sharding: data-parallel over batch B=128 across 8 cores (16 batches/core),
weights replicated. No collectives.

All matmuls run as float32r (TF32-like, 1 cycle/row on the PE at N>=256
vs 4 for fp32); PSUM accumulation stays fp32. Measured end-to-end rel
err vs the fp64 oracle is ~1e-5..1e-4.

Device layout (per core), everything f-major so the contraction dim sits on
SBUF partitions:
  xiT   [F=512, BL*256]  columns grouped 256 per batch (255 real + 1 zero pad)
  x0T   [F=512, BL=16]
  w1bT  [F=512, H=1024]  (= W1[:, F:].T)
  w1aT  [F=512, H=1024]  (= W1[:, :F].T)
  w2T   [H=1024, F=512]  (= W2.T)
  w0T   [F=512, F=512]   (= W0.T)
Output res [BL=16, F=512] per core (batch-major); host concatenates.
"""

import os
import numpy as np
from contextlib import ExitStack

import concourse.bass as bass
import concourse.tile as tile
from concourse import bacc, mybir
from concourse.bass_utils import run_bass_kernel_spmd

N_CORES = 8
B, N, F, H = 128, 256, 512, 1024
BL = B // N_CORES          # 16 batches per core
NI = N - 1                 # 255 real columns per batch
NP = 256                   # padded columns per batch
NF = F // 128              # 4 f-chunks
NH = H // 128              # 8 h-tiles
QUADS = BL // 4            # 4 batch-quads; per quad psum tile [128, 4*256]

F32 = mybir.dt.float32
F32R = mybir.dt.float32r

USE_F32R = os.environ.get("KB_NO_F32R", "") == ""
# How many of the 32 quad-tiles take the fused-ACT consumer path
# (bias+tanh+sum in per-batch activation calls); the rest take the DVE
# path. Balances ACT ~= DVE busy time.
N_ACT_PATH = int(os.environ.get("KB_NACT", "13"))


def _build_kernel():
    nc = bacc.Bacc(
        "TRN2", target_bir_lowering=False, debug=False, num_devices=N_CORES
    )

    MMDT = F32R if USE_F32R else F32
    xiT = nc.dram_tensor("xiT", [F, BL * NP], MMDT, kind="ExternalInput").ap()
    x0T = nc.dram_tensor("x0T", [F, BL], MMDT, kind="ExternalInput").ap()
    w1bT = nc.dram_tensor("w1bT", [F, H], MMDT, kind="ExternalInput").ap()
    w1aT = nc.dram_tensor("w1aT", [F, H], MMDT, kind="ExternalInput").ap()
    w2T = nc.dram_tensor("w2T", [H, F], MMDT, kind="ExternalInput").ap()
    w0T = nc.dram_tensor("w0T", [F, F], MMDT, kind="ExternalInput").ap()
    res = nc.dram_tensor("res", [BL, F], F32, kind="ExternalOutput").ap()

    with tile.TileContext(nc) as tc:
        with ExitStack() as ctx:
            _kernel_body(ctx, tc, xiT, x0T, w1bT, w1aT, w2T, w0T, res)

    nc.compile()
    return nc


def _kernel_body(ctx, tc, xiT, x0T, w1bT, w1aT, w2T, w0T, res):
    nc = tc.nc
    Tanh = mybir.ActivationFunctionType.Tanh
    MMDT = F32R if USE_F32R else F32

    wpool = ctx.enter_context(tc.tile_pool(name="weights", bufs=1))

    def load(name, dram, rows, width, dt):
        tiles = []
        for c in range(rows // 128):
            t = wpool.tile([128, width], dt, tag=f"{name}_{c}", name=f"{name}_{c}")
            nc.sync.dma_start(t[:], dram[c * 128 : (c + 1) * 128, :])
            tiles.append(t)
        return tiles

    # DMA issue order = first-need order.
    x0_sb = load("x0", x0T, F, BL, MMDT)
    w1a_sb = load("w1a", w1aT, F, H, MMDT)
    w1b_sb = load("w1b", w1bT, F, H, MMDT)
    xi_sb = load("xi", xiT, F, BL * NP, MMDT)
    w2_sb = load("w2", w2T, H, F, MMDT)
    w0_sb = load("w0", w0T, F, F, MMDT)

    h0_sb = [
        wpool.tile([128, BL], F32, tag=f"h0_{h}", name=f"h0_{h}")
        for h in range(NH)
    ]
    S_sb = [
        wpool.tile([128, BL], MMDT, tag=f"S_{h}", name=f"S_{h}")
        for h in range(NH)
    ]

    # One PSUM pool; every tile shares the tag so slots recycle.
    # Slot size = max tile = [128, 4*NP] f32 = 2 banks; 4 bufs = 8 banks.
    ppool = ctx.enter_context(tc.tile_pool(name="ps", bufs=4, space="PSUM"))
    itpool = ctx.enter_context(tc.tile_pool(name="it", bufs=4))

    # ---- Phase 0: PE warm-up during the DMA lead-in ----
    # The PE sits idle for the first ~15us while inputs stream from HBM;
    # HAM then holds it at 1.2GHz for the first ~3.4us of real work and
    # re-throttles after every stall. A stream of dummy matmuls on zeros
    # (no DMA dependency) keeps the activity monitor warm so real matmuls
    # issue at 2.4GHz from the start.
    # Plain fp32 dummies (4 cyc/row -> ~850ns each warm): ~18 cover the
    # ~15us DMA window. (f32r here trips a walrus ISA check on the memset.)
    warm_n = int(os.environ.get("KB_WARM", "18"))
    if warm_n:
        wz = wpool.tile([128, 512], F32, tag="warmz", name="warmz")
        nc.vector.memset(wz[:], 0.0)
        pw = ppool.tile([128, 512], F32, tag="ps", name="pwarm")
        for _ in range(warm_n):
            nc.tensor.matmul(pw[:], wz[:, :128], wz[:], start=True, stop=True)

    # ---- Phase 1: h0[h, b] = sum_f W1a[h, f] * x0[b, f] ----
    for h in range(NH):
        ph = ppool.tile([128, BL], F32, tag="ps", name=f"ph0_{h}")
        for f in range(NF):
            nc.tensor.matmul(
                ph[:],
                w1a_sb[f][:, h * 128 : (h + 1) * 128],
                x0_sb[f][:],
                start=(f == 0),
                stop=(f == NF - 1),
            )
        nc.vector.tensor_copy(h0_sb[h][:], ph[:])

    # ---- Phase 2: hi matmul + bias + tanh + segmented reduce ----
    def act_path(idx):
        return (idx * N_ACT_PATH) // 32 != ((idx + 1) * N_ACT_PATH) // 32

    def consume(h, q, pb):
        if act_path(h * QUADS + q):
            # fused: tanh(psum + h0) with free-dim sum into S, per batch
            for bl in range(4):
                b = q * 4 + bl
                scratch = itpool.tile(
                    [128, NI], F32, tag="scratch", name=f"sc_{h}_{q}_{bl}"
                )
                with nc.allow_low_precision(
                    reason="S in f32r so it can feed the f32r output matmul"
                ):
                    nc.scalar.activation(
                        scratch[:],
                        pb[:, bl * NP : bl * NP + NI],
                        Tanh,
                        bias=h0_sb[h][:, b : b + 1],
                        accum_out=S_sb[h][:, b : b + 1],
                    )
        else:
            it = itpool.tile([128, 4 * NP], F32, tag="it", name=f"it_{h}_{q}")
            for bl in range(4):
                b = q * 4 + bl
                nc.vector.tensor_scalar_add(
                    it[:, bl * NP : bl * NP + NI],
                    pb[:, bl * NP : bl * NP + NI],
                    h0_sb[h][:, b : b + 1],
                )
            view = it[:].rearrange("p (b n) -> p b n", b=4)[:, :, :NI]
            nc.scalar.activation(view, view, Tanh)
            with nc.allow_low_precision(
                reason="S in f32r so it can feed the f32r output matmul"
            ):
                nc.vector.reduce_sum(
                    S_sb[h][:, q * 4 : (q + 1) * 4],
                    view,
                    axis=mybir.AxisListType.X,
                )

    for h in range(NH):
        for wave in range(QUADS // 2):
            qs = (2 * wave, 2 * wave + 1)
            pbs = {
                q: ppool.tile([128, 4 * NP], F32, tag="ps", name=f"pb_{h}_{q}")
                for q in qs
            }
            for f in range(NF):
                lhsT = w1b_sb[f][:, h * 128 : (h + 1) * 128]
                for q in qs:
                    for bk in range(2):
                        cols = slice(
                            q * 4 * NP + bk * 512, q * 4 * NP + (bk + 1) * 512
                        )
                        nc.tensor.matmul(
                            pbs[q][:, bk * 512 : (bk + 1) * 512],
                            lhsT,
                            xi_sb[f][:, cols],
                            start=(f == 0),
                            stop=(f == NF - 1),
                        )
            for q in qs:
                consume(h, q, pbs[q])

    # ---- Phase 3 (flipped): res[b, g] = sum_h S[h,b] W2T[h,g]
    #                                   + sum_f x0T[f,b] W0T[f,g] ----
    # b (=16) is the PE's M dim; N=512 streams. 12 matmuls total.
    # W0 term first: it has no S dependency, so it runs while the last
    # wave's consumers are still producing S.
    po = ppool.tile([BL, F], F32, tag="ps", name="po")
    for f in range(NF):
        nc.tensor.matmul(
            po[:], x0_sb[f][:], w0_sb[f][:], start=(f == 0), stop=False
        )
    for h in range(NH):
        nc.tensor.matmul(
            po[:], S_sb[h][:], w2_sb[h][:], start=False, stop=(h == NH - 1)
        )
    rt = itpool.tile([BL, F], F32, tag="rt", name="rt")
    nc.vector.tensor_copy(rt[:], po[:])
    nc.sync.dma_start(res[:], rt[:])


_NC_CACHE = {}


def _get_nc():
    key = ("v4", USE_F32R, N_ACT_PATH, os.environ.get("KB_WARM", "70"))
    if key not in _NC_CACHE:
        _NC_CACHE[key] = _build_kernel()
    return _NC_CACHE[key]


def _make_in_maps(x, W1, W2, W0):
    x = np.ascontiguousarray(np.asarray(x, dtype=np.float32))
    W1 = np.asarray(W1, dtype=np.float32)
    W2 = np.asarray(W2, dtype=np.float32)
    W0 = np.asarray(W0, dtype=np.float32)

    w1aT = np.ascontiguousarray(W1[:, :F].T)  # [F, H]
    w1bT = np.ascontiguousarray(W1[:, F:].T)  # [F, H]
    w2T = np.ascontiguousarray(W2.T)          # [H, F]
    w0T = np.ascontiguousarray(W0.T)          # [F, F]

    in_maps = []
    for i in range(N_CORES):
        xc = x[i * BL : (i + 1) * BL]               # [BL, N, F]
        x0T = np.ascontiguousarray(xc[:, 0, :].T)   # [F, BL]
        pad = np.zeros((BL, NP, F), dtype=np.float32)
        pad[:, :NI, :] = xc[:, 1:, :]
        xiT = np.ascontiguousarray(pad.reshape(BL * NP, F).T)  # [F, BL*NP]
        in_maps.append(
            {
                "xiT": xiT,
                "x0T": x0T,
                "w1bT": w1bT,
                "w1aT": w1aT,
                "w2T": w2T,
                "w0T": w0T,
            }
        )
    return in_maps


def _gather(results):
    out = np.empty((B, F), dtype=np.float32)
    for i in range(N_CORES):
        out[i * BL : (i + 1) * BL] = results[i]["res"]
    return out


def kernel(x, W1, W2, W0):
    nc = _get_nc()
    in_maps = _make_in_maps(x, W1, W2, W0)
    res = run_bass_kernel_spmd(nc, in_maps, list(range(N_CORES)))
    return _gather(res.results)


def kernel_profiled(x, W1, W2, W0, **trace_kwargs):
    """Like kernel() but with NTFF profiling; returns (out, exec_time_ns)."""
    nc = _get_nc()
    in_maps = _make_in_maps(x, W1, W2, W0)
    res = run_bass_kernel_spmd(
        nc, in_maps, list(range(N_CORES)), trace=True, **trace_kwargs
    )
    return _gather(res.results), res.exec_time_ns


# revision 14
# speedup vs baseline: 1.5383x; 1.1706x over previous
"""Trainium2 Bass kernel for nn_Attention_39934605918652.

res[b] = W0 @ x0[b] + sum_{n=1..N-1} W2 @ tanh(W1a @ x0[b] + W1b @ x[b,n])

Key algebraic optimization: W2 does not depend on n, so
    sum_n W2 @ tanh(...) = W2 @ (sum_n tanh(...))
which removes the second big matmul (only a [B,H]x[H,F] remains).

Sharding: data-parallel over batch B=128 across 8 cores (16 batches/core),
weights replicated. No collectives.

All matmuls run as float32r (TF32-like, 1 cycle/row on the PE at N>=256
vs 4 for fp32); PSUM accumulation stays fp32. Measured end-to-end rel
err vs the fp64 oracle is ~1e-5..1e-4.

Device layout (per core), everything f-major so the contraction dim sits on
SBUF partitions:
  xiT   [F=512, BL*256]  columns grouped 256 per batch (255 real + 1 zero pad)
  x0T   [F=512, BL=16]
  w1bT  [F=512, H=1024]  (= W1[:, F:].T)
  w1aT  [F=512, H=1024]  (= W1[:, :F].T)
  w2T   [H=1024, F=512]  (= W2.T)
  w0T   [F=512, F=512]   (= W0.T)
Output res [BL=16, F=512] per core (batch-major); host concatenates.
"""

import os
import numpy as np
from contextlib import ExitStack

import concourse.bass as bass
import concourse.tile as tile
from concourse import bacc, mybir
from concourse.bass_utils import run_bass_kernel_spmd

N_CORES = 8
B, N, F, H = 128, 256, 512, 1024
BL = B // N_CORES          # 16 batches per core
NI = N - 1                 # 255 real columns per batch
NP = 256                   # padded columns per batch
NF = F // 128              # 4 f-chunks
NH = H // 128              # 8 h-tiles
QUADS = BL // 4            # 4 batch-quads; per quad psum tile [128, 4*256]

F32 = mybir.dt.float32
F32R = mybir.dt.float32r
BF16 = mybir.dt.bfloat16


def _dtypes():
    """(dtype for xi/w1b, dtype for x0/w1a/w2/w0/S)."""
    if not USE_F32R:
        return F32, F32
    if KB_DT == "bf16all":
        return BF16, BF16
    if KB_DT == "bf16xi":
        return BF16, F32R
    return F32R, F32R

USE_F32R = os.environ.get("KB_NO_F32R", "") == ""
# KB_DT: "f32r" (default), "bf16xi" (xi+w1b in bf16), "bf16all" (all matmul
# operands bf16). bf16 halves DMA bytes and the PE's SBUF read bandwidth
# (which otherwise contends with concurrent DMA writes) at a precision cost.
KB_DT = os.environ.get("KB_DT", "f32r")
# How many of the 32 quad-tiles take the fused-ACT consumer path
# (bias+tanh+sum in per-batch activation calls); the rest take the DVE
# path. Balances ACT ~= DVE busy time.
N_ACT_PATH = int(os.environ.get("KB_NACT", "13"))


def _build_kernel():
    nc = bacc.Bacc(
        "TRN2", target_bir_lowering=False, debug=False, num_devices=N_CORES
    )

    XIDT, WDT = _dtypes()
    xiT = nc.dram_tensor("xiT", [F, BL * NP], XIDT, kind="ExternalInput").ap()
    x0T = nc.dram_tensor("x0T", [F, BL], WDT, kind="ExternalInput").ap()
    w1bT = nc.dram_tensor("w1bT", [F, H], XIDT, kind="ExternalInput").ap()
    w1aT = nc.dram_tensor("w1aT", [F, H], WDT, kind="ExternalInput").ap()
    w2T = nc.dram_tensor("w2T", [H, F], WDT, kind="ExternalInput").ap()
    w0T = nc.dram_tensor("w0T", [F, F], WDT, kind="ExternalInput").ap()
    res = nc.dram_tensor("res", [BL, F], F32, kind="ExternalOutput").ap()

    with tile.TileContext(nc) as tc:
        with ExitStack() as ctx:
            _kernel_body(ctx, tc, xiT, x0T, w1bT, w1aT, w2T, w0T, res)

    nc.compile()
    return nc


def _kernel_body(ctx, tc, xiT, x0T, w1bT, w1aT, w2T, w0T, res):
    nc = tc.nc
    Tanh = mybir.ActivationFunctionType.Tanh
    XIDT, WDT = _dtypes()

    wpool = ctx.enter_context(tc.tile_pool(name="weights", bufs=1))

    def load(name, dram, rows, width, dt):
        tiles = []
        for c in range(rows // 128):
            t = wpool.tile([128, width], dt, tag=f"{name}_{c}", name=f"{name}_{c}")
            nc.sync.dma_start(t[:], dram[c * 128 : (c + 1) * 128, :])
            tiles.append(t)
        return tiles

    # DMA issue order = first-need order.
    x0_sb = load("x0", x0T, F, BL, WDT)
    w1a_sb = load("w1a", w1aT, F, H, WDT)
    w1b_sb = load("w1b", w1bT, F, H, XIDT)
    xi_sb = load("xi", xiT, F, BL * NP, XIDT)
    w2_sb = load("w2", w2T, H, F, WDT)
    w0_sb = load("w0", w0T, F, F, WDT)

    h0_sb = [
        wpool.tile([128, BL], F32, tag=f"h0_{h}", name=f"h0_{h}")
        for h in range(NH)
    ]
    S_sb = [
        wpool.tile([128, BL], WDT, tag=f"S_{h}", name=f"S_{h}")
        for h in range(NH)
    ]

    # One PSUM pool; every tile shares the tag so slots recycle.
    # Slot size = max tile = [128, 4*NP] f32 = 2 banks; 4 bufs = 8 banks.
    ppool = ctx.enter_context(tc.tile_pool(name="ps", bufs=4, space="PSUM"))
    itpool = ctx.enter_context(tc.tile_pool(name="it", bufs=4))

    # ---- Phase 0: PE warm-up during the DMA lead-in ----
    # The PE sits idle for the first ~15us while inputs stream from HBM;
    # HAM then holds it at 1.2GHz for the first ~3.4us of real work and
    # re-throttles after every stall. A stream of dummy matmuls on zeros
    # (no DMA dependency) keeps the activity monitor warm so real matmuls
    # issue at 2.4GHz from the start.
    # Plain fp32 dummies (4 cyc/row -> ~850ns each warm): ~18 cover the
    # ~15us DMA window. (f32r here trips a walrus ISA check on the memset.)
    warm_n = int(os.environ.get("KB_WARM", "18"))
    if warm_n:
        wz = wpool.tile([128, 512], F32, tag="warmz", name="warmz")
        nc.vector.memset(wz[:], 0.0)
        pw = ppool.tile([128, 512], F32, tag="ps", name="pwarm")
        for _ in range(warm_n):
            nc.tensor.matmul(pw[:], wz[:, :128], wz[:], start=True, stop=True)

    # ---- Phase 1: h0[h, b] = sum_f W1a[h, f] * x0[b, f] ----
    for h in range(NH):
        ph = ppool.tile([128, BL], F32, tag="ps", name=f"ph0_{h}")
        for f in range(NF):
            nc.tensor.matmul(
                ph[:],
                w1a_sb[f][:, h * 128 : (h + 1) * 128],
                x0_sb[f][:],
                start=(f == 0),
                stop=(f == NF - 1),
            )
        nc.vector.tensor_copy(h0_sb[h][:], ph[:])

    # ---- Phase 2: hi matmul + bias + tanh + segmented reduce ----
    def act_path(idx):
        return (idx * N_ACT_PATH) // 32 != ((idx + 1) * N_ACT_PATH) // 32

    def consume(h, q, pb):
        if act_path(h * QUADS + q):
            # fused: tanh(psum + h0) with free-dim sum into S, per batch
            for bl in range(4):
                b = q * 4 + bl
                scratch = itpool.tile(
                    [128, NI], F32, tag="scratch", name=f"sc_{h}_{q}_{bl}"
                )
                with nc.allow_low_precision(
                    reason="S in f32r so it can feed the f32r output matmul"
                ):
                    nc.scalar.activation(
                        scratch[:],
                        pb[:, bl * NP : bl * NP + NI],
                        Tanh,
                        bias=h0_sb[h][:, b : b + 1],
                        accum_out=S_sb[h][:, b : b + 1],
                    )
        else:
            it = itpool.tile([128, 4 * NP], F32, tag="it", name=f"it_{h}_{q}")
            for bl in range(4):
                b = q * 4 + bl
                nc.vector.tensor_scalar_add(
                    it[:, bl * NP : bl * NP + NI],
                    pb[:, bl * NP : bl * NP + NI],
                    h0_sb[h][:, b : b + 1],
                )
            view = it[:].rearrange("p (b n) -> p b n", b=4)[:, :, :NI]
            nc.scalar.activation(view, view, Tanh)
            with nc.allow_low_precision(
                reason="S in f32r so it can feed the f32r output matmul"
            ):
                nc.vector.reduce_sum(
                    S_sb[h][:, q * 4 : (q + 1) * 4],
                    view,
                    axis=mybir.AxisListType.X,
                )

    for h in range(NH):
        for wave in range(QUADS // 2):
            qs = (2 * wave, 2 * wave + 1)
            pbs = {
                q: ppool.tile([128, 4 * NP], F32, tag="ps", name=f"pb_{h}_{q}")
                for q in qs
            }
            for f in range(NF):
                lhsT = w1b_sb[f][:, h * 128 : (h + 1) * 128]
                for q in qs:
                    for bk in range(2):
                        cols = slice(
                            q * 4 * NP + bk * 512, q * 4 * NP + (bk + 1) * 512
                        )
                        nc.tensor.matmul(
                            pbs[q][:, bk * 512 : (bk + 1) * 512],
                            lhsT,
                            xi_sb[f][:, cols],
                            start=(f == 0),
                            stop=(f == NF - 1),
                        )
            for q in qs:
                consume(h, q, pbs[q])

    # ---- Phase 3 (flipped): res[b, g] = sum_h S[h,b] W2T[h,g]
    #                                   + sum_f x0T[f,b] W0T[f,g] ----
    # b (=16) is the PE's M dim; N=512 streams. 12 matmuls total.
    # W0 term first: it has no S dependency, so it runs while the last
    # wave's consumers are still producing S.
    po = ppool.tile([BL, F], F32, tag="ps", name="po")
    for f in range(NF):
        nc.tensor.matmul(
            po[:], x0_sb[f][:], w0_sb[f][:], start=(f == 0), stop=False
        )
    for h in range(NH):
        nc.tensor.matmul(
            po[:], S_sb[h][:], w2_sb[h][:], start=False, stop=(h == NH - 1)
        )
    rt = itpool.tile([BL, F], F32, tag="rt", name="rt")
    nc.vector.tensor_copy(rt[:], po[:])
    nc.sync.dma_start(res[:], rt[:])


_NC_CACHE = {}


def _get_nc():
    key = ("v4", USE_F32R, N_ACT_PATH, os.environ.get("KB_WARM", "70"))
    if key not in _NC_CACHE:
        _NC_CACHE[key] = _build_kernel()
    return _NC_CACHE[key]


def _np_dt(dt):
    import ml_dtypes
    return ml_dtypes.bfloat16 if dt == BF16 else np.float32


def _make_in_maps(x, W1, W2, W0):
    xidt, wdt = _dtypes()
    np_xi, np_w = _np_dt(xidt), _np_dt(wdt)
    x = np.ascontiguousarray(np.asarray(x, dtype=np.float32))
    W1 = np.asarray(W1, dtype=np.float32)
    W2 = np.asarray(W2, dtype=np.float32)
    W0 = np.asarray(W0, dtype=np.float32)

    w1aT = np.ascontiguousarray(W1[:, :F].T).astype(np_w)   # [F, H]
    w1bT = np.ascontiguousarray(W1[:, F:].T).astype(np_xi)  # [F, H]
    w2T = np.ascontiguousarray(W2.T).astype(np_w)           # [H, F]
    w0T = np.ascontiguousarray(W0.T).astype(np_w)           # [F, F]

    in_maps = []
    for i in range(N_CORES):
        xc = x[i * BL : (i + 1) * BL]               # [BL, N, F]
        x0T = np.ascontiguousarray(xc[:, 0, :].T).astype(np_w)   # [F, BL]
        pad = np.zeros((BL, NP, F), dtype=np.float32)
        pad[:, :NI, :] = xc[:, 1:, :]
        xiT = np.ascontiguousarray(pad.reshape(BL * NP, F).T).astype(np_xi)
        in_maps.append(
            {
                "xiT": xiT,
                "x0T": x0T,
                "w1bT": w1bT,
                "w1aT": w1aT,
                "w2T": w2T,
                "w0T": w0T,
            }
        )
    return in_maps


def _gather(results):
    out = np.empty((B, F), dtype=np.float32)
    for i in range(N_CORES):
        out[i * BL : (i + 1) * BL] = results[i]["res"]
    return out


def kernel(x, W1, W2, W0):
    nc = _get_nc()
    in_maps = _make_in_maps(x, W1, W2, W0)
    res = run_bass_kernel_spmd(nc, in_maps, list(range(N_CORES)))
    return _gather(res.results)


def kernel_profiled(x, W1, W2, W0, **trace_kwargs):
    """Like kernel() but with NTFF profiling; returns (out, exec_time_ns)."""
    nc = _get_nc()
    in_maps = _make_in_maps(x, W1, W2, W0)
    res = run_bass_kernel_spmd(
        nc, in_maps, list(range(N_CORES)), trace=True, **trace_kwargs
    )
    return _gather(res.results), res.exec_time_ns


# revision 16
# speedup vs baseline: 1.5862x; 1.0311x over previous
"""Trainium2 Bass kernel for nn_Attention_39934605918652.

res[b] = W0 @ x0[b] + sum_{n=1..N-1} W2 @ tanh(W1a @ x0[b] + W1b @ x[b,n])

Key algebraic optimization: W2 does not depend on n, so
    sum_n W2 @ tanh(...) = W2 @ (sum_n tanh(...))
which removes the second big matmul (only a [B,H]x[H,F] remains).

Sharding: data-parallel over batch B=128 across 8 cores (16 batches/core),
weights replicated. No collectives.

All matmuls run as float32r (TF32-like, 1 cycle/row on the PE at N>=256
vs 4 for fp32); PSUM accumulation stays fp32. Measured end-to-end rel
err vs the fp64 oracle is ~1e-5..1e-4.

Device layout (per core), everything f-major so the contraction dim sits on
SBUF partitions:
  xiT   [F=512, BL*256]  columns grouped 256 per batch (255 real + 1 zero pad)
  x0T   [F=512, BL=16]
  w1bT  [F=512, H=1024]  (= W1[:, F:].T)
  w1aT  [F=512, H=1024]  (= W1[:, :F].T)
  w2T   [H=1024, F=512]  (= W2.T)
  w0T   [F=512, F=512]   (= W0.T)
Output res [BL=16, F=512] per core (batch-major); host concatenates.
"""

import os
import numpy as np
from contextlib import ExitStack

import concourse.bass as bass
import concourse.tile as tile
from concourse import bacc, mybir
from concourse.bass_utils import run_bass_kernel_spmd

N_CORES = 8
B, N, F, H = 128, 256, 512, 1024
BL = B // N_CORES          # 16 batches per core
NI = N - 1                 # 255 real columns per batch
NP = 256                   # padded columns per batch
NF = F // 128              # 4 f-chunks
NH = H // 128              # 8 h-tiles
QUADS = BL // 4            # 4 batch-quads; per quad psum tile [128, 4*256]

F32 = mybir.dt.float32
F32R = mybir.dt.float32r
BF16 = mybir.dt.bfloat16


def _dtypes():
    """(dtype for xi/w1b, dtype for x0/w1a/w2/w0/S)."""
    if not USE_F32R:
        return F32, F32
    if KB_DT == "bf16all":
        return BF16, BF16
    if KB_DT == "bf16xi":
        return BF16, F32R
    return F32R, F32R

USE_F32R = os.environ.get("KB_NO_F32R", "") == ""
# KB_DT: "f32r" (default), "bf16xi" (xi+w1b in bf16), "bf16all" (all matmul
# operands bf16). bf16 halves DMA bytes and the PE's SBUF read bandwidth
# (which otherwise contends with concurrent DMA writes) at a precision cost.
KB_DT = os.environ.get("KB_DT", "bf16xi")
# How many of the 32 quad-tiles take the fused-ACT consumer path
# (bias+tanh+sum in per-batch activation calls); the rest take the DVE
# path. Balances ACT ~= DVE busy time.
N_ACT_PATH = int(os.environ.get("KB_NACT", "13"))


def _build_kernel():
    nc = bacc.Bacc(
        "TRN2", target_bir_lowering=False, debug=False, num_devices=N_CORES
    )

    XIDT, WDT = _dtypes()
    xiT = nc.dram_tensor("xiT", [F, BL * NP], XIDT, kind="ExternalInput").ap()
    x0T = nc.dram_tensor("x0T", [F, BL], WDT, kind="ExternalInput").ap()
    w1bT = nc.dram_tensor("w1bT", [F, H], XIDT, kind="ExternalInput").ap()
    w1aT = nc.dram_tensor("w1aT", [F, H], WDT, kind="ExternalInput").ap()
    w2T = nc.dram_tensor("w2T", [H, F], WDT, kind="ExternalInput").ap()
    w0T = nc.dram_tensor("w0T", [F, F], WDT, kind="ExternalInput").ap()
    res = nc.dram_tensor("res", [BL, F], F32, kind="ExternalOutput").ap()

    with tile.TileContext(nc) as tc:
        with ExitStack() as ctx:
            _kernel_body(ctx, tc, xiT, x0T, w1bT, w1aT, w2T, w0T, res)

    nc.compile()
    return nc


def _kernel_body(ctx, tc, xiT, x0T, w1bT, w1aT, w2T, w0T, res):
    nc = tc.nc
    Tanh = mybir.ActivationFunctionType.Tanh
    XIDT, WDT = _dtypes()

    wpool = ctx.enter_context(tc.tile_pool(name="weights", bufs=1))

    def load(name, dram, rows, width, dt):
        tiles = []
        for c in range(rows // 128):
            t = wpool.tile([128, width], dt, tag=f"{name}_{c}", name=f"{name}_{c}")
            nc.sync.dma_start(t[:], dram[c * 128 : (c + 1) * 128, :])
            tiles.append(t)
        return tiles

    # DMA issue order = first-need order.
    x0_sb = load("x0", x0T, F, BL, WDT)
    w1a_sb = load("w1a", w1aT, F, H, WDT)
    w1b_sb = load("w1b", w1bT, F, H, XIDT)
    # xi as 8 half-column tiles, all c0 halves DMA'd before any c1 half:
    # wave-major compute below starts on c0 while c1 still streams.
    HC = BL * NP // 2
    xi_sb = [[None, None] for _ in range(NF)]
    for c in range(2):
        for f in range(NF):
            t = wpool.tile([128, HC], XIDT, tag=f"xi_{f}_{c}", name=f"xi_{f}_{c}")
            nc.sync.dma_start(
                t[:], xiT[f * 128 : (f + 1) * 128, c * HC : (c + 1) * HC]
            )
            xi_sb[f][c] = t
    w2_sb = load("w2", w2T, H, F, WDT)
    w0_sb = load("w0", w0T, F, F, WDT)

    h0_sb = [
        wpool.tile([128, BL], F32, tag=f"h0_{h}", name=f"h0_{h}")
        for h in range(NH)
    ]
    S_sb = [
        wpool.tile([128, BL], WDT, tag=f"S_{h}", name=f"S_{h}")
        for h in range(NH)
    ]

    # One PSUM pool; every tile shares the tag so slots recycle.
    # Slot size = max tile = [128, 4*NP] f32 = 2 banks; 4 bufs = 8 banks.
    ppool = ctx.enter_context(tc.tile_pool(name="ps", bufs=4, space="PSUM"))
    itpool = ctx.enter_context(tc.tile_pool(name="it", bufs=4))

    # ---- Phase 0: PE warm-up during the DMA lead-in ----
    # The PE sits idle for the first ~15us while inputs stream from HBM;
    # HAM then holds it at 1.2GHz for the first ~3.4us of real work and
    # re-throttles after every stall. A stream of dummy matmuls on zeros
    # (no DMA dependency) keeps the activity monitor warm so real matmuls
    # issue at 2.4GHz from the start.
    # Plain fp32 dummies (4 cyc/row -> ~850ns each warm): ~18 cover the
    # ~15us DMA window. (f32r here trips a walrus ISA check on the memset.)
    warm_n = int(os.environ.get("KB_WARM", "18"))
    if warm_n:
        wz = wpool.tile([128, 512], F32, tag="warmz", name="warmz")
        nc.vector.memset(wz[:], 0.0)
        pw = ppool.tile([128, 512], F32, tag="ps", name="pwarm")
        for _ in range(warm_n):
            nc.tensor.matmul(pw[:], wz[:, :128], wz[:], start=True, stop=True)

    # ---- Phase 1: h0[h, b] = sum_f W1a[h, f] * x0[b, f] ----
    for h in range(NH):
        ph = ppool.tile([128, BL], F32, tag="ps", name=f"ph0_{h}")
        for f in range(NF):
            nc.tensor.matmul(
                ph[:],
                w1a_sb[f][:, h * 128 : (h + 1) * 128],
                x0_sb[f][:],
                start=(f == 0),
                stop=(f == NF - 1),
            )
        nc.vector.tensor_copy(h0_sb[h][:], ph[:])

    # ---- Phase 2: hi matmul + bias + tanh + segmented reduce ----
    def act_path(idx):
        return (idx * N_ACT_PATH) // 32 != ((idx + 1) * N_ACT_PATH) // 32

    _consume_counter = [0]

    def consume(h, q, pb):
        idx = _consume_counter[0]
        _consume_counter[0] += 1
        if act_path(idx):
            # fused: tanh(psum + h0) with free-dim sum into S, per batch
            for bl in range(4):
                b = q * 4 + bl
                scratch = itpool.tile(
                    [128, NI], F32, tag="scratch", name=f"sc_{h}_{q}_{bl}"
                )
                with nc.allow_low_precision(
                    reason="S in f32r so it can feed the f32r output matmul"
                ):
                    nc.scalar.activation(
                        scratch[:],
                        pb[:, bl * NP : bl * NP + NI],
                        Tanh,
                        bias=h0_sb[h][:, b : b + 1],
                        accum_out=S_sb[h][:, b : b + 1],
                    )
        else:
            it = itpool.tile([128, 4 * NP], F32, tag="it", name=f"it_{h}_{q}")
            for bl in range(4):
                b = q * 4 + bl
                nc.vector.tensor_scalar_add(
                    it[:, bl * NP : bl * NP + NI],
                    pb[:, bl * NP : bl * NP + NI],
                    h0_sb[h][:, b : b + 1],
                )
            view = it[:].rearrange("p (b n) -> p b n", b=4)[:, :, :NI]
            nc.scalar.activation(view, view, Tanh)
            with nc.allow_low_precision(
                reason="S in f32r so it can feed the f32r output matmul"
            ):
                nc.vector.reduce_sum(
                    S_sb[h][:, q * 4 : (q + 1) * 4],
                    view,
                    axis=mybir.AxisListType.X,
                )

    for wave in range(QUADS // 2):
        for h in range(NH):
            qs = (2 * wave, 2 * wave + 1)
            pbs = {
                q: ppool.tile([128, 4 * NP], F32, tag="ps", name=f"pb_{h}_{q}")
                for q in qs
            }
            for f in range(NF):
                lhsT = w1b_sb[f][:, h * 128 : (h + 1) * 128]
                for q in qs:
                    for bk in range(2):
                        cols = slice(
                            (q % 2) * 4 * NP + bk * 512,
                            (q % 2) * 4 * NP + (bk + 1) * 512,
                        )
                        nc.tensor.matmul(
                            pbs[q][:, bk * 512 : (bk + 1) * 512],
                            lhsT,
                            xi_sb[f][wave][:, cols],
                            start=(f == 0),
                            stop=(f == NF - 1),
                        )
            for q in qs:
                consume(h, q, pbs[q])

    # ---- Phase 3 (flipped): res[b, g] = sum_h S[h,b] W2T[h,g]
    #                                   + sum_f x0T[f,b] W0T[f,g] ----
    # b (=16) is the PE's M dim; N=512 streams. 12 matmuls total.
    # W0 term first: it has no S dependency, so it runs while the last
    # wave's consumers are still producing S.
    po = ppool.tile([BL, F], F32, tag="ps", name="po")
    for f in range(NF):
        nc.tensor.matmul(
            po[:], x0_sb[f][:], w0_sb[f][:], start=(f == 0), stop=False
        )
    for h in range(NH):
        nc.tensor.matmul(
            po[:], S_sb[h][:], w2_sb[h][:], start=False, stop=(h == NH - 1)
        )
    rt = itpool.tile([BL, F], F32, tag="rt", name="rt")
    nc.vector.tensor_copy(rt[:], po[:])
    nc.sync.dma_start(res[:], rt[:])


_NC_CACHE = {}


def _get_nc():
    key = ("v4", USE_F32R, N_ACT_PATH, os.environ.get("KB_WARM", "70"))
    if key not in _NC_CACHE:
        _NC_CACHE[key] = _build_kernel()
    return _NC_CACHE[key]


def _np_dt(dt):
    import ml_dtypes
    return ml_dtypes.bfloat16 if dt == BF16 else np.float32


def _make_in_maps(x, W1, W2, W0):
    xidt, wdt = _dtypes()
    np_xi, np_w = _np_dt(xidt), _np_dt(wdt)
    x = np.ascontiguousarray(np.asarray(x, dtype=np.float32))
    W1 = np.asarray(W1, dtype=np.float32)
    W2 = np.asarray(W2, dtype=np.float32)
    W0 = np.asarray(W0, dtype=np.float32)

    w1aT = np.ascontiguousarray(W1[:, :F].T).astype(np_w)   # [F, H]
    w1bT = np.ascontiguousarray(W1[:, F:].T).astype(np_xi)  # [F, H]
    w2T = np.ascontiguousarray(W2.T).astype(np_w)           # [H, F]
    w0T = np.ascontiguousarray(W0.T).astype(np_w)           # [F, F]

    in_maps = []
    for i in range(N_CORES):
        xc = x[i * BL : (i + 1) * BL]               # [BL, N, F]
        x0T = np.ascontiguousarray(xc[:, 0, :].T).astype(np_w)   # [F, BL]
        pad = np.zeros((BL, NP, F), dtype=np.float32)
        pad[:, :NI, :] = xc[:, 1:, :]
        xiT = np.ascontiguousarray(pad.reshape(BL * NP, F).T).astype(np_xi)
        in_maps.append(
            {
                "xiT": xiT,
                "x0T": x0T,
                "w1bT": w1bT,
                "w1aT": w1aT,
                "w2T": w2T,
                "w0T": w0T,
            }
        )
    return in_maps


def _gather(results):
    out = np.empty((B, F), dtype=np.float32)
    for i in range(N_CORES):
        out[i * BL : (i + 1) * BL] = results[i]["res"]
    return out


def kernel(x, W1, W2, W0):
    nc = _get_nc()
    in_maps = _make_in_maps(x, W1, W2, W0)
    res = run_bass_kernel_spmd(nc, in_maps, list(range(N_CORES)))
    return _gather(res.results)


def kernel_profiled(x, W1, W2, W0, **trace_kwargs):
    """Like kernel() but with NTFF profiling; returns (out, exec_time_ns)."""
    nc = _get_nc()
    in_maps = _make_in_maps(x, W1, W2, W0)
    res = run_bass_kernel_spmd(
        nc, in_maps, list(range(N_CORES)), trace=True, **trace_kwargs
    )
    return _gather(res.results), res.exec_time_ns
